# revision 39
# baseline (speedup 1.0000x reference)
"""DynamicConv1D Trainium2 kernel.

Reference computation (per batch b):
  dw = conv1d(x, W, pad=3) + b            # [O*I*K, T] dynamic weights
  dw = softmax(dw.reshape(O,I,K,T)/sqrt(K), axis=K)
  y[o,t] = sum_{i,k} x[i, t+k-3] * dw[o,i,k,t]

Sharding: 8 cores = 4 batches x 2 halves of O (16 out-channels each).
Each core gets x[b] plus its half of the (rearranged) conv weights and
computes y[b, half*16:(half+1)*16, :]. No collectives; the host scatters
inputs and concatenates outputs.

Per-core layout (t-tile = 128 positions on partitions):
  conv as matmul: dw[t, (k,o,i)] = sum_{(j,c)} X1[(j,c), t] * W'[(j,c), (k,o,i)]
    X1[(j,c), u] = x[c, u+j-3]  (im2col built host-side, bf16; ones row
    appended so the bias rides as an extra W' row; 1/sqrt(K) folded in).
  PSUM is split into two multi-bank tiles (4 banks + 3 banks) so the exp
  drains as two wide ACTIVATEs instead of seven narrow ones.
  Per tile (t on partitions, (k,o,i) on free):
    e  = exp(dw)                ScalarE, 2 wide ACTs
    EX = e * x_unf              DVE (x_unf shipped pre-transposed from host)
    den/num = sum_k {e, EX}     DVE pair-add tree batched over {den,num}
                                (GpSimd shares DVE's SBUF port: offloading
                                elementwise work there halves DVE throughput,
                                so everything stays on DVE)
  Tiles are processed in groups of GRP=8; the softmax tail is batched per
  group so the ScalarE table set only flips exp<->reciprocal twice per group:
    r  = 1/den                  ScalarE Reciprocal, one wide ACT per group
    y1 = num * r                DVE, one wide op per group
    y[t,o] = sum_i y1           DVE pair-add tree (2x mode; tensor_reduce
                                only has a 1x uop)
"""

import numpy as np

B = 4
C = 32
K = 7
T = 4096
O_FULL = 32
OH = 16  # out-channels per core
PAD = 3
TT = 128  # t positions per tile (partition dim)
FREE = K * OH * C  # 3584, free index = k*512 + o*32 + i
SLAB = OH * C  # 512, one k-slab
CD1 = 128  # (j, c) rows for j=0..3
CD2 = 97  # (j, c) rows for j=4..6 plus ones row
CHUNK = 512  # one psum bank; FREE = 7*CHUNK
NA = 4  # chunks in psum tile A
NB = 3  # chunks in psum tile B
GRP = 8  # tiles per softmax-tail group

_prog_cache = {}


def _act_raw(nc, out, in_, func):
    """Emit an InstActivation directly (bass blocks Reciprocal in the
    activation() helper over accuracy concerns; at this kernel's 2e-2
    tolerance the HW spline is fine — verified against the reference)."""
    from concourse import mybir

    imm = lambda v: mybir.ImmediateValue(dtype=mybir.dt.float32, value=v)
    return nc.scalar.add_instruction(
        mybir.InstActivation(
            name=nc.get_next_instruction_name(),
            func=func,
            ins=[nc.scalar.lower_ap(in_), imm(0.0), imm(1.0), imm(0.0)],
            outs=[nc.scalar.lower_ap(out)],
        )
    )


def _build(t_len):
    """Build and compile the per-core Bass program for sequence length t_len."""
    import concourse.tile as tile
    from concourse import bacc, mybir

    nt = t_len // TT
    ng = nt // GRP
    nc = bacc.Bacc("TRN2", target_bir_lowering=False, debug=False, num_devices=1)
    f32 = mybir.dt.float32
    bf16 = mybir.dt.bfloat16
    Act = mybir.ActivationFunctionType

    x1a_d = nc.dram_tensor("x1a", [CD1, t_len], bf16, kind="ExternalInput").ap()
    x1b_d = nc.dram_tensor("x1b", [CD2, t_len], bf16, kind="ExternalInput").ap()
    w1_d = nc.dram_tensor("wp1", [CD1, FREE], bf16, kind="ExternalInput").ap()
    w2_d = nc.dram_tensor("wp2", [CD2, FREE], bf16, kind="ExternalInput").ap()
    x2_d = nc.dram_tensor("x2f", [TT, nt * K * C], bf16, kind="ExternalInput").ap()
    y_d = nc.dram_tensor("yout", [TT, nt * OH], f32, kind="ExternalOutput").ap()

    with tile.TileContext(nc) as tc:
        with (
            tc.tile_pool(name="const", bufs=1) as cpool,
            tc.tile_pool(name="ep", bufs=4) as epool,
            tc.tile_pool(name="tree", bufs=2) as tpool,
            tc.tile_pool(name="grp", bufs=2) as gpool,
            tc.tile_pool(name="psum", bufs=1, space="PSUM") as ppool,
        ):
            x1a_bf = cpool.tile([CD1, t_len], bf16, tag="x1abf")
            x1b_bf = cpool.tile([CD2, t_len], bf16, tag="x1bbf")
            w1_bf = cpool.tile([CD1, FREE], bf16, tag="w1bf")
            w2_bf = cpool.tile([CD2, FREE], bf16, tag="w2bf")
            x2_bf = cpool.tile([TT, nt, K * C], bf16, tag="x2bf")
            y_sb = cpool.tile([TT, nt * OH], f32, tag="ysb")

            # Input loads split across the sync/scalar/gpsimd DMA queues,
            # first-needed data first.
            x2_flat = x2_bf[:].rearrange("p n q -> p (n q)")
            h = t_len // 2
            q = t_len // 8
            gx = GRP * K * C
            # All loads go upfront: the Tile scheduler makes compute wait on
            # each queue's cumulative DMA completion count at its program
            # position, so in-loop prefetches stall the tiles behind them
            # every group, while upfront loads cost one startup wait that
            # overlaps the pipeline fill. Ordering within the queues puts
            # tile-0-critical data (w chunks in chunk order) first.
            nc.sync.dma_start(x1a_bf[:, 0:TT], x1a_d[:, 0:TT])
            nc.gpsimd.dma_start(x1b_bf[:, 0:TT], x1b_d[:, 0:TT])
            for ci in range(K):
                cs = slice(ci * CHUNK, (ci + 1) * CHUNK)
                (nc.sync if ci % 2 == 0 else nc.scalar).dma_start(
                    w1_bf[:, cs], w1_d[:, cs]
                )
                (nc.gpsimd if ci % 2 == 0 else nc.sync).dma_start(
                    w2_bf[:, cs], w2_d[:, cs]
                )
            nc.scalar.dma_start(x2_flat[:, 0:gx], x2_d[:, 0:gx])
            nc.sync.dma_start(x1a_bf[:, TT:q], x1a_d[:, TT:q])
            nc.gpsimd.dma_start(x1b_bf[:, TT:q], x1b_d[:, TT:q])
            nc.sync.dma_start(x1a_bf[:, q:h], x1a_d[:, q:h])
            nc.gpsimd.dma_start(x1b_bf[:, q:h], x1b_d[:, q:h])
            nc.gpsimd.dma_start(x2_flat[:, gx : 2 * gx], x2_d[:, gx : 2 * gx])
            nc.sync.dma_start(x1a_bf[:, h:], x1a_d[:, h:])
            nc.gpsimd.dma_start(x1b_bf[:, h:], x1b_d[:, h:])
            nc.sync.dma_start(x2_flat[:, 2 * gx :], x2_d[:, 2 * gx :])

            def emit_tail(dnum, gbase, gs):
                """Batched softmax tail for one group of gs tiles."""
                ys = slice(gbase * OH, (gbase + gs) * OH)
                r = gpool.tile([TT, GRP, SLAB], bf16, tag="r", name="r")[:, 0:gs]
                _act_raw(nc, r[:], dnum[:, :, 0], Act.Reciprocal)
                y1 = gpool.tile([TT, GRP, SLAB], bf16, tag="y1", name="y1")[:, 0:gs]
                nc.vector.tensor_mul(y1[:], dnum[:, :, 1], r[:])
                # i-sum as a pair-add tree: tensor_reduce only has a 1x uop,
                # the bf16 adds run at 2x.
                y1v = y1[:].rearrange("p g (o i) -> p g o i", o=OH)
                u16 = gpool.tile([TT, GRP, OH, 16], bf16, tag="u16", name="u16")[:, 0:gs]
                nc.vector.tensor_add(u16[:], y1v[:, :, :, 0:16], y1v[:, :, :, 16:32])
                u8 = gpool.tile([TT, GRP, OH, 8], bf16, tag="u8", name="u8")[:, 0:gs]
                nc.vector.tensor_add(u8[:], u16[:, :, :, 0:8], u16[:, :, :, 8:16])
                u4 = gpool.tile([TT, GRP, OH, 4], bf16, tag="u4", name="u4")[:, 0:gs]
                nc.vector.tensor_add(u4[:], u8[:, :, :, 0:4], u8[:, :, :, 4:8])
                u2 = gpool.tile([TT, GRP, OH, 2], bf16, tag="u2", name="u2")[:, 0:gs]
                nc.vector.tensor_add(u2[:], u4[:, :, :, 0:2], u4[:, :, :, 2:4])
                nc.vector.tensor_add(
                    y_sb[:, ys].rearrange("p (g o) -> p g o", g=gs),
                    u2[:, :, :, 0],
                    u2[:, :, :, 1],
                )
                nc.sync.dma_start(y_d[:, ys], y_sb[:, ys])

            # Last group split in two so the final softmax tail is shorter.
            sizes = [GRP] * (ng - 1) + [GRP // 2, GRP - GRP // 2]
            tbase = 0
            pending = None
            for gi, gs in enumerate(sizes):
                dnum = gpool.tile([TT, GRP, 2, SLAB], bf16, tag="dnum", name="dnum")[:, 0:gs]

                for ti in range(gs):
                    tt = tbase + ti
                    # The previous group's tail is emitted one tile into this
                    # group; bulk loads are prefetched mid-group, far enough
                    # ahead that the tiles emitted after them never stall.
                    if ti == 1 and pending is not None:
                        emit_tail(*pending)
                        pending = None
                    t0 = tt * TT
                    xa = x1a_bf[:, t0 : t0 + TT]
                    xb = x1b_bf[:, t0 : t0 + TT]

                    eex = epool.tile([TT, 2, FREE], bf16, tag="eex")
                    e = eex[:, 0]
                    ex = eex[:, 1]

                    pa = ppool.tile([TT, NA * CHUNK], f32, tag="pa", name="pa")
                    for ci in range(NA):
                        cs = slice(ci * CHUNK, (ci + 1) * CHUNK)
                        nc.tensor.matmul(
                            pa[:, cs], xa, w1_bf[:, cs], start=True, stop=False
                        )
                        nc.tensor.matmul(
                            pa[:, cs], xb, w2_bf[:, cs], start=False, stop=True
                        )
                    nc.scalar.activation(e[:, 0 : NA * CHUNK], pa[:], Act.Exp)

                    pb = ppool.tile([TT, NB * CHUNK], f32, tag="pb", name="pb")
                    for ci in range(NA, K):
                        cs = slice(ci * CHUNK, (ci + 1) * CHUNK)
                        ps = slice((ci - NA) * CHUNK, (ci - NA + 1) * CHUNK)
                        nc.tensor.matmul(
                            pb[:, ps], xa, w1_bf[:, cs], start=True, stop=False
                        )
                        nc.tensor.matmul(
                            pb[:, ps], xb, w2_bf[:, cs], start=False, stop=True
                        )
                    nc.scalar.activation(e[:, NA * CHUNK : FREE], pb[:], Act.Exp)

                    # EX = e * x_unf broadcast over o; split on the psum A/B
                    # boundary so EX-A overlaps the second exp.
                    e4 = e.rearrange("p (k o i) -> p k o i", k=K, o=OH)
                    x24 = (
                        x2_bf[:, tt]
                        .rearrange("p (k i) -> p k i", k=K)
                        .unsqueeze(2)
                        .broadcast_to([TT, K, OH, C])
                    )
                    ex4 = ex.rearrange("p (k o i) -> p k o i", k=K, o=OH)
                    nc.vector.tensor_mul(ex4, e4, x24)

                    # k-sum trees for den (over e) and num (over EX), batched
                    # as one wide op per level via the [TT, 2, ...] eex view.
                    pairs = eex[:, :, 0 : 6 * SLAB].rearrange(
                        "p s (x k q) -> p s x k q", x=3, k=2
                    )
                    t1 = tpool.tile([TT, 2, 3, SLAB], bf16, tag="t1")
                    nc.vector.tensor_add(t1[:], pairs[:, :, :, 0], pairs[:, :, :, 1])
                    t2 = tpool.tile([TT, 2, SLAB], bf16, tag="t2")
                    nc.vector.tensor_add(t2[:], t1[:, :, 0], t1[:, :, 1])
                    t3 = tpool.tile([TT, 2, SLAB], bf16, tag="t3")
                    sl6 = eex[:].rearrange("p s (k q) -> p s k q", k=K)[:, :, 6]
                    nc.vector.tensor_add(t3[:], t1[:, :, 2], sl6)
                    # GpSimd compute is a net loss here (its SBUF port is
                    # shared with DVE and halves DVE throughput while active),
                    # so the whole tree stays on DVE.
                    nc.vector.tensor_add(dnum[:, ti], t2[:], t3[:])

                pending = (dnum, tbase, gs)
                tbase += gs
            emit_tail(*pending)

    nc.compile()
    return nc


def _prep_inputs(x, W, b):
    """Host-side scatter: per-core input dicts (pure layout/slicing)."""
    import ml_dtypes

    bf = ml_dtypes.bfloat16
    scale = np.float32(1.0 / np.sqrt(K))
    halves = []
    for h in range(2):
        Wh = W[h * OH * C * K : (h + 1) * OH * C * K]  # [OH*C*K, C, K]
        # rows (j,c) -> j*32+c ; cols (k,o,i) -> k*512 + o*32 + i
        Wp = (
            Wh.reshape(OH, C, K, C, K).transpose(4, 3, 2, 0, 1).reshape(K * C, FREE)
            * scale
        )
        bh = (
            b[h * OH * C * K : (h + 1) * OH * C * K]
            .reshape(OH, C, K)
            .transpose(2, 0, 1)
            .reshape(FREE)
            * scale
        )
        w1 = np.ascontiguousarray(Wp[:CD1])
        w2 = np.ascontiguousarray(np.concatenate([Wp[CD1:], bh[None, :]], axis=0))
        halves.append((w1.astype(bf), w2.astype(bf)))

    t_len = x.shape[-1]
    nt = t_len // TT
    x1s = []
    for bi in range(B):
        xp = np.zeros((C, t_len + 2 * PAD), dtype=np.float32)
        xp[:, PAD : PAD + t_len] = x[bi]
        x1a = np.empty((CD1, t_len), dtype=np.float32)
        x1b = np.empty((CD2, t_len), dtype=np.float32)
        for j in range(K):
            tgt, r0 = (x1a, j * C) if j < 4 else (x1b, (j - 4) * C)
            tgt[r0 : r0 + C] = xp[:, j : j + t_len]
        x1b[CD2 - 1] = 1.0
        # x_unf, pre-transposed: x2f[tp, tt*224 + k*32 + i] = x[i, tt*128+tp+k-3]
        rows = np.empty((K * C, t_len), dtype=np.float32)
        for k in range(K):
            rows[k * C : (k + 1) * C] = xp[:, k : k + t_len]
        x2f = np.ascontiguousarray(
            rows.reshape(K * C, nt, TT).transpose(2, 1, 0).reshape(TT, nt * K * C)
        )
        x1s.append((x1a.astype(bf), x1b.astype(bf), x2f.astype(bf)))

    in_maps = []
    for core in range(8):
        bi, h = divmod(core, 2)
        w1, w2 = halves[h]
        x1a, x1b, x2f = x1s[bi]
        in_maps.append({"x1a": x1a, "x1b": x1b, "wp1": w1, "wp2": w2, "x2f": x2f})
    return in_maps


def _assemble(results, t_len):
    """Gather per-core [TT, nt*OH] outputs into [B, O_FULL, t_len]."""
    nt = t_len // TT
    y = np.empty((B, O_FULL, t_len), dtype=np.float32)
    for core, res in enumerate(results):
        bi, h = divmod(core, 2)
        arr = res["yout"].reshape(TT, nt, OH)  # [tp, tt, o]
        y[bi, h * OH : (h + 1) * OH, :] = arr.transpose(2, 1, 0).reshape(OH, t_len)
    return y


def _run(x, W, b, trace=False, trace_cores=None):
    from concourse.bass_utils import run_bass_kernel_spmd
    from concourse.bass_interp import get_hw_module

    t_len = x.shape[-1]
    key = ("prog", t_len)
    if key not in _prog_cache:
        nc = _build(t_len)
        nc.m = get_hw_module(nc.m)
        _prog_cache[key] = nc
    nc = _prog_cache[key]

    in_maps = _prep_inputs(x, W, b)
    res = run_bass_kernel_spmd(
        nc,
        in_maps,
        core_ids=list(range(8)),
        trace=trace,
        trace_cores=trace_cores,
    )
    return _assemble(res.results, t_len), res


def kernel(x, W, b):
    y, _ = _run(np.asarray(x), np.asarray(W), np.asarray(b))
    return y


# revision 40
# speedup vs baseline: 1.0119x; 1.0119x over previous
"""DynamicConv1D Trainium2 kernel.

Reference computation (per batch b):
  dw = conv1d(x, W, pad=3) + b            # [O*I*K, T] dynamic weights
  dw = softmax(dw.reshape(O,I,K,T)/sqrt(K), axis=K)
  y[o,t] = sum_{i,k} x[i, t+k-3] * dw[o,i,k,t]

Sharding: 8 cores = 4 batches x 2 halves of O (16 out-channels each).
Each core gets x[b] plus its half of the (rearranged) conv weights and
computes y[b, half*16:(half+1)*16, :]. No collectives; the host scatters
inputs and concatenates outputs.

Per-core layout (t-tile = 128 positions on partitions):
  conv as matmul: dw[t, (k,o,i)] = sum_{(j,c)} X1[(j,c), t] * W'[(j,c), (k,o,i)]
    X1[(j,c), u] = x[c, u+j-3]  (im2col built host-side, bf16; ones row
    appended so the bias rides as an extra W' row; 1/sqrt(K) folded in).
  PSUM is split into two multi-bank tiles (4 banks + 3 banks) so the exp
  drains as two wide ACTIVATEs instead of seven narrow ones.
  Per tile (t on partitions, (k,o,i) on free):
    e  = exp(dw)                ScalarE, 2 wide ACTs
    EX = e * x_unf              DVE (x_unf shipped pre-transposed from host)
    den/num = sum_k {e, EX}     DVE pair-add tree batched over {den,num}
                                (GpSimd shares DVE's SBUF port: offloading
                                elementwise work there halves DVE throughput,
                                so everything stays on DVE)
  Tiles are processed in groups of GRP=8; the softmax tail is batched per
  group so the ScalarE table set only flips exp<->reciprocal twice per group:
    r  = 1/den                  ScalarE Reciprocal, one wide ACT per group
    y1 = num * r                DVE, one wide op per group
    y[t,o] = sum_i y1           DVE pair-add tree (2x mode; tensor_reduce
                                only has a 1x uop)
"""

import numpy as np

B = 4
C = 32
K = 7
T = 4096
O_FULL = 32
OH = 16  # out-channels per core
PAD = 3
TT = 128  # t positions per tile (partition dim)
FREE = K * OH * C  # 3584, free index = k*512 + o*32 + i
SLAB = OH * C  # 512, one k-slab
CD1 = 128  # (j, c) rows for j=0..3
CD2 = 97  # (j, c) rows for j=4..6 plus ones row
CHUNK = 512  # one psum bank; FREE = 7*CHUNK
NA = 4  # chunks in psum tile A
NB = 3  # chunks in psum tile B
GRP = 8  # tiles per softmax-tail group

_prog_cache = {}


def _act_raw(nc, out, in_, func):
    """Emit an InstActivation directly (bass blocks Reciprocal in the
    activation() helper over accuracy concerns; at this kernel's 2e-2
    tolerance the HW spline is fine — verified against the reference)."""
    from concourse import mybir

    imm = lambda v: mybir.ImmediateValue(dtype=mybir.dt.float32, value=v)
    return nc.scalar.add_instruction(
        mybir.InstActivation(
            name=nc.get_next_instruction_name(),
            func=func,
            ins=[nc.scalar.lower_ap(in_), imm(0.0), imm(1.0), imm(0.0)],
            outs=[nc.scalar.lower_ap(out)],
        )
    )


def _build(t_len):
    """Build and compile the per-core Bass program for sequence length t_len."""
    import concourse.tile as tile
    from concourse import bacc, mybir

    nt = t_len // TT
    ng = nt // GRP
    nc = bacc.Bacc("TRN2", target_bir_lowering=False, debug=False, num_devices=1)
    f32 = mybir.dt.float32
    bf16 = mybir.dt.bfloat16
    Act = mybir.ActivationFunctionType

    x1a_d = nc.dram_tensor("x1a", [CD1, t_len], bf16, kind="ExternalInput").ap()
    x1b_d = nc.dram_tensor("x1b", [CD2, t_len], bf16, kind="ExternalInput").ap()
    w1_d = nc.dram_tensor("wp1", [CD1, FREE], bf16, kind="ExternalInput").ap()
    w2_d = nc.dram_tensor("wp2", [CD2, FREE], bf16, kind="ExternalInput").ap()
    x2_d = nc.dram_tensor("x2f", [TT, nt * K * C], bf16, kind="ExternalInput").ap()
    y_d = nc.dram_tensor("yout", [TT, nt * OH], f32, kind="ExternalOutput").ap()

    with tile.TileContext(nc) as tc:
        with (
            tc.tile_pool(name="const", bufs=1) as cpool,
            tc.tile_pool(name="ep", bufs=4) as epool,
            tc.tile_pool(name="tree", bufs=2) as tpool,
            tc.tile_pool(name="grp", bufs=2) as gpool,
            tc.tile_pool(name="psum", bufs=1, space="PSUM") as ppool,
        ):
            x1a_bf = cpool.tile([CD1, t_len], bf16, tag="x1abf")
            x1b_bf = cpool.tile([CD2, t_len], bf16, tag="x1bbf")
            w1_bf = cpool.tile([CD1, FREE], bf16, tag="w1bf")
            w2_bf = cpool.tile([CD2, FREE], bf16, tag="w2bf")
            x2_bf = cpool.tile([TT, nt, K * C], bf16, tag="x2bf")
            y_sb = cpool.tile([TT, nt * OH], f32, tag="ysb")

            # Input loads split across the sync/scalar/gpsimd DMA queues,
            # first-needed data first.
            x2_flat = x2_bf[:].rearrange("p n q -> p (n q)")
            h = t_len // 2
            q = t_len // 8
            gx = GRP * K * C
            # All loads go upfront: the Tile scheduler makes compute wait on
            # each queue's cumulative DMA completion count at its program
            # position, so in-loop prefetches stall the tiles behind them
            # every group, while upfront loads cost one startup wait that
            # overlaps the pipeline fill. Ordering within the queues puts
            # tile-0-critical data (w chunks in chunk order) first.
            nc.sync.dma_start(x1a_bf[:, 0:TT], x1a_d[:, 0:TT])
            nc.gpsimd.dma_start(x1b_bf[:, 0:TT], x1b_d[:, 0:TT])
            for ci in range(K):
                cs = slice(ci * CHUNK, (ci + 1) * CHUNK)
                (nc.sync if ci % 2 == 0 else nc.scalar).dma_start(
                    w1_bf[:, cs], w1_d[:, cs]
                )
                nc.gpsimd.dma_start(w2_bf[:, cs], w2_d[:, cs])
            nc.scalar.dma_start(x2_flat[:, 0:gx], x2_d[:, 0:gx])
            nc.sync.dma_start(x1a_bf[:, TT:q], x1a_d[:, TT:q])
            nc.gpsimd.dma_start(x1b_bf[:, TT:q], x1b_d[:, TT:q])
            nc.sync.dma_start(x1a_bf[:, q:h], x1a_d[:, q:h])
            nc.gpsimd.dma_start(x1b_bf[:, q:h], x1b_d[:, q:h])
            nc.gpsimd.dma_start(x2_flat[:, gx : 2 * gx], x2_d[:, gx : 2 * gx])
            nc.sync.dma_start(x1a_bf[:, h:], x1a_d[:, h:])
            nc.gpsimd.dma_start(x1b_bf[:, h:], x1b_d[:, h:])
            nc.sync.dma_start(x2_flat[:, 2 * gx :], x2_d[:, 2 * gx :])

            def emit_tail(dnum, gbase, gs):
                """Batched softmax tail for one group of gs tiles."""
                ys = slice(gbase * OH, (gbase + gs) * OH)
                r = gpool.tile([TT, GRP, SLAB], bf16, tag="r", name="r")[:, 0:gs]
                _act_raw(nc, r[:], dnum[:, :, 0], Act.Reciprocal)
                y1 = gpool.tile([TT, GRP, SLAB], bf16, tag="y1", name="y1")[:, 0:gs]
                nc.vector.tensor_mul(y1[:], dnum[:, :, 1], r[:])
                # i-sum as a pair-add tree: tensor_reduce only has a 1x uop,
                # the bf16 adds run at 2x.
                y1v = y1[:].rearrange("p g (o i) -> p g o i", o=OH)
                u16 = gpool.tile([TT, GRP, OH, 16], bf16, tag="u16", name="u16")[:, 0:gs]
                nc.vector.tensor_add(u16[:], y1v[:, :, :, 0:16], y1v[:, :, :, 16:32])
                u8 = gpool.tile([TT, GRP, OH, 8], bf16, tag="u8", name="u8")[:, 0:gs]
                nc.vector.tensor_add(u8[:], u16[:, :, :, 0:8], u16[:, :, :, 8:16])
                u4 = gpool.tile([TT, GRP, OH, 4], bf16, tag="u4", name="u4")[:, 0:gs]
                nc.vector.tensor_add(u4[:], u8[:, :, :, 0:4], u8[:, :, :, 4:8])
                u2 = gpool.tile([TT, GRP, OH, 2], bf16, tag="u2", name="u2")[:, 0:gs]
                nc.vector.tensor_add(u2[:], u4[:, :, :, 0:2], u4[:, :, :, 2:4])
                nc.vector.tensor_add(
                    y_sb[:, ys].rearrange("p (g o) -> p g o", g=gs),
                    u2[:, :, :, 0],
                    u2[:, :, :, 1],
                )
                nc.sync.dma_start(y_d[:, ys], y_sb[:, ys])

            # Last group split in two so the final softmax tail is shorter.
            sizes = [GRP] * (ng - 1) + [GRP // 2, GRP - GRP // 2]
            tbase = 0
            pending = None
            for gi, gs in enumerate(sizes):
                dnum = gpool.tile([TT, GRP, 2, SLAB], bf16, tag="dnum", name="dnum")[:, 0:gs]

                for ti in range(gs):
                    tt = tbase + ti
                    # The previous group's tail is emitted one tile into this
                    # group; bulk loads are prefetched mid-group, far enough
                    # ahead that the tiles emitted after them never stall.
                    if ti == 1 and pending is not None:
                        emit_tail(*pending)
                        pending = None
                    t0 = tt * TT
                    xa = x1a_bf[:, t0 : t0 + TT]
                    xb = x1b_bf[:, t0 : t0 + TT]

                    eex = epool.tile([TT, 2, FREE], bf16, tag="eex")
                    e = eex[:, 0]
                    ex = eex[:, 1]

                    pa = ppool.tile([TT, NA * CHUNK], f32, tag="pa", name="pa")
                    for ci in range(NA):
                        cs = slice(ci * CHUNK, (ci + 1) * CHUNK)
                        nc.tensor.matmul(
                            pa[:, cs], xa, w1_bf[:, cs], start=True, stop=False
                        )
                        nc.tensor.matmul(
                            pa[:, cs], xb, w2_bf[:, cs], start=False, stop=True
                        )
                    nc.scalar.activation(e[:, 0 : NA * CHUNK], pa[:], Act.Exp)

                    pb = ppool.tile([TT, NB * CHUNK], f32, tag="pb", name="pb")
                    for ci in range(NA, K):
                        cs = slice(ci * CHUNK, (ci + 1) * CHUNK)
                        ps = slice((ci - NA) * CHUNK, (ci - NA + 1) * CHUNK)
                        nc.tensor.matmul(
                            pb[:, ps], xa, w1_bf[:, cs], start=True, stop=False
                        )
                        nc.tensor.matmul(
                            pb[:, ps], xb, w2_bf[:, cs], start=False, stop=True
                        )
                    nc.scalar.activation(e[:, NA * CHUNK : FREE], pb[:], Act.Exp)

                    # EX = e * x_unf broadcast over o; split on the psum A/B
                    # boundary so EX-A overlaps the second exp.
                    e4 = e.rearrange("p (k o i) -> p k o i", k=K, o=OH)
                    x24 = (
                        x2_bf[:, tt]
                        .rearrange("p (k i) -> p k i", k=K)
                        .unsqueeze(2)
                        .broadcast_to([TT, K, OH, C])
                    )
                    ex4 = ex.rearrange("p (k o i) -> p k o i", k=K, o=OH)
                    nc.vector.tensor_mul(ex4, e4, x24)

                    # k-sum trees for den (over e) and num (over EX), batched
                    # as one wide op per level via the [TT, 2, ...] eex view.
                    pairs = eex[:, :, 0 : 6 * SLAB].rearrange(
                        "p s (x k q) -> p s x k q", x=3, k=2
                    )
                    t1 = tpool.tile([TT, 2, 3, SLAB], bf16, tag="t1")
                    nc.vector.tensor_add(t1[:], pairs[:, :, :, 0], pairs[:, :, :, 1])
                    t2 = tpool.tile([TT, 2, SLAB], bf16, tag="t2")
                    nc.vector.tensor_add(t2[:], t1[:, :, 0], t1[:, :, 1])
                    t3 = tpool.tile([TT, 2, SLAB], bf16, tag="t3")
                    sl6 = eex[:].rearrange("p s (k q) -> p s k q", k=K)[:, :, 6]
                    nc.vector.tensor_add(t3[:], t1[:, :, 2], sl6)
                    # GpSimd compute is a net loss here (its SBUF port is
                    # shared with DVE and halves DVE throughput while active),
                    # so the whole tree stays on DVE.
                    nc.vector.tensor_add(dnum[:, ti], t2[:], t3[:])

                pending = (dnum, tbase, gs)
                tbase += gs
            emit_tail(*pending)

    nc.compile()
    return nc


def _prep_inputs(x, W, b):
    """Host-side scatter: per-core input dicts (pure layout/slicing)."""
    import ml_dtypes

    bf = ml_dtypes.bfloat16
    scale = np.float32(1.0 / np.sqrt(K))
    halves = []
    for h in range(2):
        Wh = W[h * OH * C * K : (h + 1) * OH * C * K]  # [OH*C*K, C, K]
        # rows (j,c) -> j*32+c ; cols (k,o,i) -> k*512 + o*32 + i
        Wp = (
            Wh.reshape(OH, C, K, C, K).transpose(4, 3, 2, 0, 1).reshape(K * C, FREE)
            * scale
        )
        bh = (
            b[h * OH * C * K : (h + 1) * OH * C * K]
            .reshape(OH, C, K)
            .transpose(2, 0, 1)
            .reshape(FREE)
            * scale
        )
        w1 = np.ascontiguousarray(Wp[:CD1])
        w2 = np.ascontiguousarray(np.concatenate([Wp[CD1:], bh[None, :]], axis=0))
        halves.append((w1.astype(bf), w2.astype(bf)))

    t_len = x.shape[-1]
    nt = t_len // TT
    x1s = []
    for bi in range(B):
        xp = np.zeros((C, t_len + 2 * PAD), dtype=np.float32)
        xp[:, PAD : PAD + t_len] = x[bi]
        x1a = np.empty((CD1, t_len), dtype=np.float32)
        x1b = np.empty((CD2, t_len), dtype=np.float32)
        for j in range(K):
            tgt, r0 = (x1a, j * C) if j < 4 else (x1b, (j - 4) * C)
            tgt[r0 : r0 + C] = xp[:, j : j + t_len]
        x1b[CD2 - 1] = 1.0
        # x_unf, pre-transposed: x2f[tp, tt*224 + k*32 + i] = x[i, tt*128+tp+k-3]
        rows = np.empty((K * C, t_len), dtype=np.float32)
        for k in range(K):
            rows[k * C : (k + 1) * C] = xp[:, k : k + t_len]
        x2f = np.ascontiguousarray(
            rows.reshape(K * C, nt, TT).transpose(2, 1, 0).reshape(TT, nt * K * C)
        )
        x1s.append((x1a.astype(bf), x1b.astype(bf), x2f.astype(bf)))

    in_maps = []
    for core in range(8):
        bi, h = divmod(core, 2)
        w1, w2 = halves[h]
        x1a, x1b, x2f = x1s[bi]
        in_maps.append({"x1a": x1a, "x1b": x1b, "wp1": w1, "wp2": w2, "x2f": x2f})
    return in_maps


def _assemble(results, t_len):
    """Gather per-core [TT, nt*OH] outputs into [B, O_FULL, t_len]."""
    nt = t_len // TT
    y = np.empty((B, O_FULL, t_len), dtype=np.float32)
    for core, res in enumerate(results):
        bi, h = divmod(core, 2)
        arr = res["yout"].reshape(TT, nt, OH)  # [tp, tt, o]
        y[bi, h * OH : (h + 1) * OH, :] = arr.transpose(2, 1, 0).reshape(OH, t_len)
    return y


def _run(x, W, b, trace=False, trace_cores=None):
    from concourse.bass_utils import run_bass_kernel_spmd
    from concourse.bass_interp import get_hw_module

    t_len = x.shape[-1]
    key = ("prog", t_len)
    if key not in _prog_cache:
        nc = _build(t_len)
        nc.m = get_hw_module(nc.m)
        _prog_cache[key] = nc
    nc = _prog_cache[key]

    in_maps = _prep_inputs(x, W, b)
    res = run_bass_kernel_spmd(
        nc,
        in_maps,
        core_ids=list(range(8)),
        trace=trace,
        trace_cores=trace_cores,
    )
    return _assemble(res.results, t_len), res


def kernel(x, W, b):
    y, _ = _run(np.asarray(x), np.asarray(W), np.asarray(b))
    return y
